# revision 1
# baseline (speedup 1.0000x reference)
"""Multi-head causal self-attention (B=2, N=4096, C=512, H=8, D=64) on 8 TRN2 cores.

Sharding: core = b*4 + g  (b = batch 0..1, g = head-group 0..3, 2 heads each).
Each core computes qkv^T for its 2 heads from x[b]^T, flash-style causal
attention in S^T [keys, q] layout (softmax without max-subtraction; logits are
|.| <= ~3), and a partial output projection over its 128 channels.  Host sums
the 4 partial y^T per batch and adds the bias.

The attention inner loop is software-pipelined: the AV matmuls of unit i are
emitted after the S matmuls + exp of unit i+1, so the PE streams S(i+1) while
the scalar engine exponentiates unit i.  Epilogues (softmax normalization) and
the output projection are deferred further to keep them off the critical path.
"""

import os

import numpy as np
import ml_dtypes

_CACHE: dict = {}
LAST_RESULTS = None

B, C = 2, 512
H, D = 8, 64
N = 4096
NQT = 8          # q tiles of 512
NKB = 32         # key blocks of 128
QT = 512
KB = 128


def _build():
    import concourse.bass as bass
    import concourse.bacc as bacc
    import concourse.mybir as mybir
    import concourse.tile as tile

    dt = mybir.dt
    bf = dt.bfloat16
    f32 = dt.float32
    Exp = mybir.ActivationFunctionType.Exp

    nc = bacc.Bacc("TRN2", target_bir_lowering=False)
    xt = nc.dram_tensor("xt", [C, N], bf, kind="ExternalInput")
    wq = nc.dram_tensor("wq", [C, 128], bf, kind="ExternalInput")
    wk = nc.dram_tensor("wk", [C, 128], bf, kind="ExternalInput")
    wv = nc.dram_tensor("wv", [C, 128], bf, kind="ExternalInput")
    wp = nc.dram_tensor("wp", [128, C], bf, kind="ExternalInput")
    tri = nc.dram_tensor("tri", [128, 128], bf, kind="ExternalInput")
    yt = nc.dram_tensor("yt", [C, N], f32, kind="ExternalOutput")

    with tile.TileContext(nc) as tc:
        with (
            tc.tile_pool(name="persist", bufs=1) as pp,
            tc.tile_pool(name="pf", bufs=3) as pf_pool,
            tc.tile_pool(name="pd", bufs=2) as pd_pool,
            tc.tile_pool(name="on", bufs=2) as on_pool,
            tc.tile_pool(name="bc", bufs=3) as bc_pool,
            tc.tile_pool(name="rc", bufs=2) as rc_pool,
            tc.tile_pool(name="yo", bufs=3) as yo_pool,
            tc.tile_pool(name="ps_s", bufs=3, space="PSUM") as ps_s,
            tc.tile_pool(name="ps_o", bufs=2, space="PSUM") as ps_o,
        ):
            xt_sb = pp.tile([128, 4, N], bf)
            wq_sb = pp.tile([128, 4, 128], bf)
            wk_sb = pp.tile([128, 4, 128], bf)
            wv_sb = pp.tile([128, 4, 128], bf)
            wp_sb = pp.tile([128, C], bf)
            tri_sb = pp.tile([128, 128], bf)
            qT = pp.tile([128, N], bf)
            kT = pp.tile([128, N], bf)
            v_sb = pp.tile([128, NKB, 130], bf)

            nc.gpsimd.dma_start(out=wq_sb[:, :, :], in_=wq.rearrange("(c p) f -> p c f", p=128))
            nc.gpsimd.dma_start(out=wk_sb[:, :, :], in_=wk.rearrange("(c p) f -> p c f", p=128))
            nc.gpsimd.dma_start(out=wv_sb[:, :, :], in_=wv.rearrange("(c p) f -> p c f", p=128))
            nc.gpsimd.dma_start(out=wp_sb, in_=wp[:, :])
            nc.gpsimd.dma_start(out=tri_sb, in_=tri[:, :])
            nc.vector.memset(v_sb, 1.0)

            xt_re = xt.rearrange("(c p) n -> p c n", p=128)

            def pa_qk(n, dst, wsb, with_dma):
                def piece():
                    if with_dma:
                        nc.sync.dma_start(
                            out=xt_sb[:, :, QT * n:QT * (n + 1)],
                            in_=xt_re[:, :, QT * n:QT * (n + 1)],
                        )
                    ps = ps_s.tile([128, 512], f32, tag="s", name=f"pa_{n}")
                    for c in range(4):
                        nc.tensor.matmul(
                            ps,
                            wsb[:, c, :],
                            xt_sb[:, c, QT * n:QT * (n + 1)],
                            start=(c == 0),
                            stop=(c == 3),
                        )
                    nc.vector.tensor_copy(dst[:, QT * n:QT * (n + 1)], ps)
                return piece

            def pa_v(kb):
                def piece():
                    ps = ps_s.tile([128, 512], f32, tag="s", name=f"pav_{kb}")
                    pv = ps[:, 0:128]
                    for c in range(4):
                        nc.tensor.matmul(
                            pv,
                            xt_sb[:, c, KB * kb:KB * (kb + 1)],
                            wv_sb[:, c, :],
                            start=(c == 0),
                            stop=(c == 3),
                        )
                    nc.vector.tensor_copy(
                        v_sb[:, kb, :].rearrange("p (h j) -> p h j", h=2)[:, :, 0:64],
                        pv.rearrange("p (h j) -> p h j", h=2),
                    )
                return piece

            def phase_a_pieces(n):
                return [
                    pa_qk(n, qT, wq_sb, True),
                    pa_qk(n, kT, wk_sb, False),
                    pa_v(4 * n),
                    pa_v(4 * n + 1),
                    pa_v(4 * n + 2),
                    pa_v(4 * n + 3),
                ]

            # diag slot layout keeps every matmul inside one 2KB PSUM bank:
            # r1 -> [0:384], r3 -> [384:512] (bank 0), r2 -> [512:768] (bank 1)
            offs = (0, 512, 384)
            wid = (384, 256, 128)

            psO_map = {}
            rc_map = {}
            on_map = {}
            import heapq
            deferred = []  # heap of (due_unit_index, seq, closure)
            seq_counter = [0]

            def defer(due, fn):
                heapq.heappush(deferred, (due, seq_counter[0], fn))
                seq_counter[0] += 1

            def flush(i):
                while deferred and deferred[0][0] <= i:
                    heapq.heappop(deferred)[2]()

            def get_psO(qt, h):
                key = (qt, h)
                if key not in psO_map:
                    psO_map[key] = ps_o.tile([128, 512], f32, tag="o", name=f"psO_{qt}_{h}")
                return psO_map[key]

            def make_av_full(qt, h, kbs, Pf):
                def av():
                    psO = get_psO(qt, h)
                    for j, kb in enumerate(kbs):
                        nc.tensor.matmul(
                            psO[0:65, :],
                            v_sb[:, kb, 65 * h:65 * h + 65],
                            Pf[:, 512 * j:512 * (j + 1)],
                            start=(kb == 0),
                            stop=False,
                            skip_group_check=True,
                        )
                return av

            def make_av_diag(qt, h, Pd):
                def av():
                    psO = get_psO(qt, h)
                    for r in (1, 2, 3):
                        nc.tensor.matmul(
                            psO[0:65, 128 * r:512],
                            v_sb[:, 4 * qt + r, 65 * h:65 * h + 65],
                            Pd[:, offs[r - 1]:offs[r - 1] + wid[r - 1]],
                            start=False,
                            stop=(r == 3),
                            skip_group_check=True,
                        )
                return av

            def make_epilogue(qt, h):
                def epi():
                    psO = psO_map.pop((qt, h))
                    if qt not in rc_map:
                        rc_map[qt] = rc_pool.tile([128, 1024], f32, tag="rc", name=f"rc_{qt}")
                    rc = rc_map[qt]
                    nc.vector.reciprocal(
                        out=rc[0:1, 512 * h:512 * (h + 1)],
                        in_=psO[64:65, :],
                    )
                    bch = bc_pool.tile([128, 512], f32, tag="bc")
                    nc.gpsimd.partition_broadcast(
                        out_ap=bch, in_ap=rc[0:1, 512 * h:512 * (h + 1)]
                    )
                    if qt not in on_map:
                        on_map[qt] = on_pool.tile([128, 512], bf, tag="on", name=f"on_{qt}")
                    nc.vector.tensor_mul(
                        on_map[qt][64 * h:64 * h + 64, :], psO[0:64, :], bch[0:64, :]
                    )
                return epi

            def make_proj_ob(qt, ob):
                def proj():
                    out_norm = on_map[qt]
                    psY = ps_o.tile([128, 512], f32, tag="o", name=f"psY_{qt}_{ob}")
                    nc.tensor.matmul(
                        psY,
                        wp_sb[:, 128 * ob:128 * (ob + 1)],
                        out_norm,
                        start=True,
                        stop=True,
                    )
                    y_sb = yo_pool.tile([128, 512], f32, tag="yo")
                    nc.vector.tensor_copy(y_sb, psY)
                    nc.sync.dma_start(
                        out=yt[128 * ob:128 * (ob + 1), QT * qt:QT * (qt + 1)],
                        in_=y_sb,
                    )
                    if ob == 3:
                        on_map.pop(qt)
                        rc_map.pop(qt, None)
                return proj

            ui = 0
            for piece in phase_a_pieces(0):
                piece()
            pa_pending = []
            for qt in range(NQT):
                for piece in pa_pending:
                    piece()
                pa_pending = phase_a_pieces(qt + 1) if qt + 1 < NQT else []
                for h in range(2):
                    b0 = 64 * h
                    # ---- full units: kb groups of 2 over kb = 0..4qt
                    nfull = 4 * qt + 1
                    kb = 0
                    while kb < nfull:
                        w = min(2, nfull - kb)
                        kbs = list(range(kb, kb + w))
                        psS = ps_s.tile([128, 1024], f32, tag="s")
                        for j, kbj in enumerate(kbs):
                            nc.tensor.matmul(
                                psS[:, 512 * j:512 * (j + 1)],
                                kT[b0:b0 + 64, KB * kbj:KB * (kbj + 1)],
                                qT[b0:b0 + 64, QT * qt:QT * (qt + 1)],
                                start=True,
                                stop=True,
                            )
                        Pf = pf_pool.tile([128, 1024], bf, tag="pf")
                        nc.scalar.activation(Pf[:, 0:512 * w], psS[:, 0:512 * w], Exp)
                        if kbs[-1] == 4 * qt:
                            j = w - 1
                            nc.vector.tensor_mul(
                                Pf[:, 512 * j:512 * j + 128],
                                Pf[:, 512 * j:512 * j + 128],
                                tri_sb,
                            )
                        flush(ui)
                        defer(ui + 2, make_av_full(qt, h, kbs, Pf))
                        if pa_pending:
                            pa_pending.pop(0)()
                        ui += 1
                        kb += w
                    # ---- diag unit: r = 1..3 packed [r1|r3|r2]
                    psD = ps_s.tile([128, 768], f32, tag="s")
                    for r in (1, 2, 3):
                        kbr = 4 * qt + r
                        nc.tensor.matmul(
                            psD[:, offs[r - 1]:offs[r - 1] + wid[r - 1]],
                            kT[b0:b0 + 64, KB * kbr:KB * (kbr + 1)],
                            qT[b0:b0 + 64, QT * qt + 128 * r:QT * qt + 128 * r + wid[r - 1]],
                            start=True,
                            stop=True,
                        )
                    Pd = pd_pool.tile([128, 768], bf, tag="pd")
                    nc.scalar.activation(Pd, psD, Exp)
                    for r in (1, 2, 3):
                        nc.vector.tensor_mul(
                            Pd[:, offs[r - 1]:offs[r - 1] + 128],
                            Pd[:, offs[r - 1]:offs[r - 1] + 128],
                            tri_sb,
                        )
                    flush(ui)
                    defer(ui + 2, make_av_diag(qt, h, Pd))
                    defer(ui + 4, make_epilogue(qt, h))
                    if h == 1:
                        for ob in range(4):
                            defer(ui + 6 + ob, make_proj_ob(qt, ob))
                    if pa_pending:
                        pa_pending.pop(0)()
                    ui += 1
            flush(10 ** 9)

    nc.compile()
    return nc


def kernel(x, w_qkv, w_proj, b_proj):
    global LAST_RESULTS
    from concourse.bass_utils import run_bass_kernel_spmd

    if "nc" not in _CACHE:
        _CACHE["nc"] = _build()
    nc = _CACHE["nc"]

    x = np.asarray(x)
    w_qkv = np.asarray(w_qkv)
    w_proj = np.asarray(w_proj)
    b_proj = np.asarray(b_proj)
    bf16 = ml_dtypes.bfloat16
    scale = D ** -0.5

    tri = np.triu(np.ones((128, 128), np.float32)).astype(bf16)
    in_maps = []
    for core in range(8):
        b, g = divmod(core, 4)
        xt = np.ascontiguousarray(x[b].T).astype(bf16)
        wq = np.ascontiguousarray((w_qkv[128 * g:128 * (g + 1), :].T * scale)).astype(bf16)
        wk = np.ascontiguousarray(w_qkv[C + 128 * g:C + 128 * (g + 1), :].T).astype(bf16)
        wv = np.ascontiguousarray(w_qkv[2 * C + 128 * g:2 * C + 128 * (g + 1), :].T).astype(bf16)
        wp = np.ascontiguousarray(w_proj[:, 128 * g:128 * (g + 1)].T).astype(bf16)
        in_maps.append({"xt": xt, "wq": wq, "wk": wk, "wv": wv, "wp": wp, "tri": tri})

    res = run_bass_kernel_spmd(
        nc,
        in_maps,
        core_ids=list(range(8)),
        trace=bool(os.environ.get("KERNEL_TRACE")),
    )
    LAST_RESULTS = res

    y = np.empty((B, N, C), np.float32)
    for b in range(B):
        acc = res.results[4 * b]["yt"].astype(np.float32)
        for g in range(1, 4):
            acc = acc + res.results[4 * b + g]["yt"]
        y[b] = acc.T + b_proj
    return y

